# revision 8
# baseline (speedup 1.0000x reference)
"""Trainium2 Bass kernel for the paired-view ("flip") multi-head attention module.

Full computation (reference semantics, B=2 P=2 S=1024 D=1024 H=16):
    q/k/v = Linear(x) -> [B,P,H,S,DK]
    left  = softmax(q k^T / 8 + mask) v          (same pair index)
    right = softmax(q k_flip^T / 8 + mask) v_flip (pair index swapped)
    out   = (left + 0.1*tanh(right)) @ Wo.T + bo

Sharding over 8 NeuronCores: data-parallel on B (2 groups of 4 cores),
tensor-parallel on heads within a group (4 heads/core, 256 channels).
Each core computes its heads' projections (column-parallel), full attention
for its heads over both pair views, and a row-parallel partial of the output
projection.  The host sums the 4 partials per batch and adds bo.

The kernel is paced by the ACT (scalar) engine's exp stream: 16.8M softmax
exps per core at 1 elem/cycle/lane (~147us incl. per-instruction overhead).
v2 reworks the schedule so PE/DMA stay off the exp critical path:
  - scores are computed TRANSPOSED ([k, q]); softmax denominators ride as a
    ones column in V; the two heads of a pair live on PE row-groups 0-63 /
    64-127 so their K=64 QK matmuls run concurrently.
  - lead-in: the 5 critical DMAs (wq/wk + 3 x-stages) are split across the
    two HWDGE rings (sync/scalar) + SWDGE (gpsimd) so the first exp fires
    ~app 17us; warmup matmuls sized to the DMA landing time keep the PE
    clock (HAM) at 8/8.
  - fills are balanced so every window's PE load is ~13us vs the 18.4us
    exp budget; v-projection chunks are spread over windows 0-3.
  - combine chains are DMA-free: DVE reciprocal straight off the av
    denominator row + gpsimd partition_broadcast (no srs/rrow round trips).
  - the LAST window's right-path AV accumulates incrementally in PSUM as
    exp chunks land (qb0), so the tail after the final exp is only:
    qb1 fp8 AVs + per-qb combine + output projection, all at warm clock,
    with output DMAs spread over all three rings.  Tail ~14us vs 52us.
"""

import numpy as np

import concourse.bass as bass
import concourse.tile as tile
from concourse import bacc, mybir
from concourse.bass_utils import run_bass_kernel_spmd

F32 = mybir.dt.float32
F32R = mybir.dt.float32r
BF16 = mybir.dt.bfloat16
FP8 = mybir.dt.float8e4
I32 = mybir.dt.int32

X_DT = BF16    # projection inputs: xT staging + Wq/Wk/Wv
QK_DT = BF16   # q/k tiles feeding the scores matmul
EXL_DT = BF16  # left-path exp(scores) + v (classic AV matmul)
EXR_DT = FP8   # right-path exp(scores) + v (fp8 DoubleRow AV matmul)
OUT_DT = BF16  # combine + Wo feeding the output projection
AF = mybir.ActivationFunctionType
OP = mybir.AluOpType
DR = mybir.MatmulPerfMode.DoubleRow

B, P, S, D, H = 2, 2, 1024, 1024, 16
DK = D // H          # 64
NCORES = 8
GROUP = 4            # cores per batch entry
NH = H // GROUP      # 4 local heads per core
CH = NH * DK         # 256 local channels
R = P * S            # 2048 rows per batch entry
KC = 8               # d_model chunks of 128
VW = 68              # padded per-head v8 block (65 used, 16B-aligned stride)
MASK_NEG = 60.0      # exp(-60) == 0 relative to any sum
WARMUP_N = 48        # PE clock-ramp matmuls covering the lead-in DMA time
SS_BUFS = 2          # scores PSUM double-buffer (4 banks)
DEBUG = False


def _emit(nc, tc, tens, out_d, dbg=None):
    from contextlib import ExitStack

    with ExitStack() as ctx:
        sb = ctx.enter_context(tc.tile_pool(name="sb", bufs=1))
        ps = ctx.enter_context(tc.tile_pool(name="ps", bufs=1, space="PSUM"))
        _body(nc, sb, ps, tens, out_d, dbg=dbg)


def _body(nc, sb, ps, tens, out_d, dbg=None):
    xq, xk, xv = tens["xq"], tens["xk"], tens["xv"]
    wq, wk, wv, wo = tens["wq"], tens["wk"], tens["wv"], tens["wo"]
    bq, bk, bv, mask = tens["bq"], tens["bk"], tens["bv"], tens["mask"]

    # ---- constants (host-packed layouts; plain contiguous DMAs) --------
    wq_sb = sb.tile([128, KC * CH], X_DT, name="wq_sb")
    wk_sb = sb.tile([128, KC * CH], X_DT, name="wk_sb")
    wv_sb = sb.tile([128, KC * CH], X_DT, name="wv_sb")
    wo_sb = sb.tile([128, 2 * D], OUT_DT, name="wo_sb")

    bq_sb = sb.tile([128, 2], F32, name="bq_sb")
    bk_sb = sb.tile([128, 2], F32, name="bk_sb")
    bv_row = sb.tile([1, CH], F32, name="bv_row")
    bv_bc = sb.tile([128, CH], F32, name="bv_bc")

    # mask as a per-key additive bias on the scores (per-partition scalar)
    mask_sb = sb.tile([128, 2 * KC], I32, name="mask_sb")
    mbias = sb.tile([128, 2 * KC], F32, name="mbias")

    # --- tiny control DMAs go on the SWDGE ring so the two HWDGE rings
    # carry only the critical bulk stages during the lead-in
    nc.gpsimd.dma_start(
        out=mask_sb[:], in_=mask[:].rearrange("pp (kc p) -> p pp kc", p=128)
    )
    nc.gpsimd.dma_start(out=bq_sb[:], in_=bq[:].rearrange("(mo p) -> p mo", p=128))
    nc.gpsimd.dma_start(out=bk_sb[:], in_=bk[:].rearrange("(mo p) -> p mo", p=128))
    nc.gpsimd.dma_start(out=bv_row[:], in_=bv[None, :])

    # --- critical bulk: weights for q/k on the two HWDGE rings
    nc.scalar.dma_start(out=wq_sb[:], in_=wq[:])
    nc.sync.dma_start(out=wk_sb[:], in_=wk[:])

    # PE warm-up fodder: dependency-free matmuls issued while the lead
    # DMAs land keep the HAM activity window busy so the PE clock is at
    # 8/8 (2.4 GHz) when real work starts.
    warm_a = sb.tile([128, 128], BF16, name="warm_a")
    warm_b = sb.tile([128, 512], BF16, name="warm_b")
    nc.vector.memset(warm_a[:], 0.0)
    nc.vector.memset(warm_b[:], 0.0)

    def warmup(n):
        wp = ps.tile([128, 512], F32, name="warm_ps", tag="ss", bufs=SS_BUFS)
        for _ in range(n):
            nc.tensor.matmul(wp[:], warm_a[:], warm_b[:], start=True, stop=True)

    # ---- projections ---------------------------------------------------
    # qT/kT: [o_local, p*S + s] in 2 tiles of 128 channels (2 heads each);
    # heads 2m / 2m+1 sit on partitions 0-63 / 64-127 of tile m
    qT = [sb.tile([128, R], QK_DT, name=f"qT{mo}") for mo in range(2)]
    kT = [sb.tile([128, R], QK_DT, name=f"kT{mo}") for mo in range(2)]
    # vb: left-path V, [r_local, rc(16) x (h(4) x 65)]; col h*65+64 = ones
    vb = sb.tile([128, 16 * NH * 65], EXL_DT, name="vb")
    nc.vector.memset(vb[:], 1.0)
    # v8: right-path V in fp8; per-head block padded to VW=68 so the
    # DoubleRow k-subtile stride (NH*VW) is 16B-aligned
    v8 = sb.tile([128, 16 * NH * VW], EXR_DT, name="v8")
    nc.vector.memset(v8[:], 1.0)

    _stage_cache = {}

    def stage(kind, rb, eng):
        """Issue the HBM->SBUF staging DMA for x-block rb (idempotent)."""
        if (kind, rb) in _stage_cache:
            return
        src_d = {"q": xq, "k": xk, "v": xv}[kind]
        st = sb.tile(
            [128, KC * 512], X_DT, name=f"st_{kind}", tag=f"st_{kind}",
            bufs=1 if kind == "v" else 2,
        )
        _stage_cache[(kind, rb)] = st
        eng.dma_start(out=st[:], in_=src_d[rb * 128 : (rb + 1) * 128, :])

    def proj_chunk(kind, rb, part):
        """Projection compute for one (x-block, channel-half); stage() must
        already have been issued for (kind, rb)."""
        w_sb = {"q": wq_sb, "k": wk_sb, "v": wv_sb}[kind]
        st = _stage_cache[(kind, rb)]
        if kind in ("q", "k"):
            mo = part
            dst, b_sb = (qT, bq_sb) if kind == "q" else (kT, bk_sb)
            pp_t = ps.tile([128, 512], F32, name="ps_proj", tag="ps_proj", bufs=2)
            for kc in range(KC):
                nc.tensor.matmul(
                    pp_t[:],
                    w_sb[:, kc * CH + mo * 128 : kc * CH + (mo + 1) * 128],
                    st[:, kc * 512 : (kc + 1) * 512],
                    start=(kc == 0),
                    stop=(kc == KC - 1),
                )
            nc.vector.tensor_scalar(
                out=dst[mo][:, rb * 512 : (rb + 1) * 512],
                in0=pp_t[:],
                scalar1=b_sb[:, mo : mo + 1],
                scalar2=None,
                op0=OP.add,
            )
        else:
            rs = part
            rc = rb * 4 + rs
            pv_t = ps.tile([128, CH], F32, name="ps_v", tag="ps_proj", bufs=2)
            for kc in range(KC):
                nc.tensor.matmul(
                    pv_t[:],
                    st[:, kc * 512 + rs * 128 : kc * 512 + (rs + 1) * 128],
                    wv_sb[:, kc * CH : (kc + 1) * CH],
                    start=(kc == 0),
                    stop=(kc == KC - 1),
                )
            for dst_t, w in ((vb, 65), (v8, VW)):
                dst_ap = dst_t[
                    :, rc * NH * w : (rc + 1) * NH * w
                ].rearrange("p (h x) -> p h x", h=NH)[:, :, 0:DK]
                nc.vector.tensor_tensor(
                    out=dst_ap,
                    in0=pv_t[:].rearrange("p (h x) -> p h x", h=NH),
                    in1=bv_bc[:].rearrange("p (h x) -> p h x", h=NH),
                    op=OP.add,
                )

    # ---- attention building blocks -------------------------------------
    comb = [sb.tile([128, R], OUT_DT, name=f"comb{kk}") for kk in range(2)]

    def qk_pair(p, m, side, exs, kcs):
        """Scores + exp for both heads of pair m, key chunks kcs.

        The two heads' matmuls are interleaved so consecutive PE
        instructions target row-groups 0-63 / 64-127 and overlap.
        """
        pp = p if side == 0 else 1 - p
        for kc in kcs:
            sss = [
                ps.tile([128, 1024], F32, name="ss", tag="ss", bufs=SS_BUFS)
                for _ in range(2)
            ]
            for qb in (0, 1):
                for hi in (0, 1):
                    po = hi * 64
                    nc.tensor.matmul(
                        sss[hi][:, qb * 512 : (qb + 1) * 512],
                        kT[m][po : po + 64, pp * S + kc * 128 : pp * S + (kc + 1) * 128],
                        qT[m][po : po + 64, p * S + qb * 512 : p * S + (qb + 1) * 512],
                        start=True,
                        stop=True,
                    )
            for hi in (0, 1):
                nc.scalar.activation(
                    exs[hi][:, kc * 1024 : (kc + 1) * 1024],
                    sss[hi][:],
                    AF.Exp,
                    bias=mbias[:, pp * KC + kc : pp * KC + kc + 1],
                    scale=0.125,
                )

    def av_left_qb(av, p, m, hi, ex_t, qb):
        """Left path: classic bf16 P @ V (lhsT = v chunk, M=65), one q half."""
        pp = p
        h = 2 * m + hi
        vbr = vb[:].rearrange("p (rc x) -> p rc x", rc=16)
        pa = ps.tile([65, 512], F32, name="ps_av", tag="ps_proj", bufs=2)
        for kc in range(KC):
            nc.tensor.matmul(
                pa[:],
                vbr[:, pp * 8 + kc, h * 65 : (h + 1) * 65],
                ex_t[:, kc * 1024 + qb * 512 : kc * 1024 + (qb + 1) * 512],
                start=(kc == 0),
                stop=(kc == KC - 1),
            )
        nc.vector.tensor_copy(av[:, qb * 512 : (qb + 1) * 512], pa[:])

    def av_right(p, m, hi, ex_t):
        """Right path: fp8 DoubleRow P @ V (2 k-chunks per matmul)."""
        pp = 1 - p
        h = 2 * m + hi
        av = sb.tile([65, S], F32, name="av", tag="avT", bufs=5)
        exr = ex_t[:].rearrange("p (k n) -> p k n", k=KC)
        v8r = v8[:].rearrange("p (rc x) -> p rc x", rc=16)
        for qb in (0, 1):
            pa = ps.tile([65, 512], F32, name="ps_av8", tag="ps_proj", bufs=2)
            for pr in range(4):
                nc.tensor.matmul(
                    pa[:],
                    v8r[:, pp * 8 + 2 * pr : pp * 8 + 2 * pr + 2, h * VW : h * VW + 65],
                    exr[:, 2 * pr : 2 * pr + 2, qb * 512 : (qb + 1) * 512],
                    start=(pr == 0),
                    stop=(pr == 3),
                    perf_mode=DR,
                )
            nc.vector.tensor_copy(av[:, qb * 512 : (qb + 1) * 512], pa[:])
        return av

    _pair = {}

    def combine(p, h, avL, avR):
        """Normalize both paths for head h; DMA-free: DVE reciprocal off the
        denominator row (64) then gpsimd broadcast."""
        rrL = sb.tile([1, S], F32, name="rrL", tag="rrow", bufs=2)
        rrR = sb.tile([1, S], F32, name="rrR", tag="rrow", bufs=2)
        nc.vector.reciprocal(rrL[:], avL[64:65, :])
        nc.vector.reciprocal(rrR[:], avR[64:65, :])

        def part2():
            po = (h % 2) * 64
            bc2 = sb.tile([64, 2 * S], F32, name="bc2", tag="bc", bufs=1)
            nc.gpsimd.partition_broadcast(bc2[:, 0:S], rrL[:])
            nc.gpsimd.partition_broadcast(bc2[:, S : 2 * S], rrR[:])
            if h % 2 == 0:
                t1p = sb.tile([128, S], F32, name="t1p", tag="t1", bufs=1)
                t2p = sb.tile([128, S], F32, name="t2p", tag="t2", bufs=1)
                _pair[(p, h // 2)] = (t1p, t2p)
            else:
                t1p, t2p = _pair[(p, h // 2)]
            nc.vector.tensor_tensor(
                out=t1p[po : po + 64, :], in0=avL[0:64, :], in1=bc2[:, 0:S],
                op=OP.mult,
            )
            nc.vector.tensor_tensor(
                out=t2p[po : po + 64, :], in0=avR[0:64, :], in1=bc2[:, S : 2 * S],
                op=OP.mult,
            )
            if h % 2 == 1:
                t3p = sb.tile([128, S], F32, name="t3p", tag="t3", bufs=1)
                nc.scalar.activation(t3p[:], t2p[:], AF.Tanh)
                nc.vector.scalar_tensor_tensor(
                    out=comb[h // 2][:, p * S : (p + 1) * S],
                    in0=t3p[:],
                    scalar=0.1,
                    in1=t1p[:],
                    op0=OP.mult,
                    op1=OP.add,
                )

        return part2

    OUT_RINGS_MID = None  # set after engines known

    def outproj_rc(p, rc, act_copy=False, ring=None):
        od = sb.tile([128, D], F32, name="od", tag="od", bufs=3)
        for ob in range(2):
            po_t = ps.tile([128, 512], F32, name="ps_o", tag="ps_proj", bufs=2)
            for kk in range(2):
                nc.tensor.matmul(
                    po_t[:],
                    comb[kk][:, p * S + rc * 128 : p * S + (rc + 1) * 128],
                    wo_sb[:, kk * D + ob * 512 : kk * D + (ob + 1) * 512],
                    start=(kk == 0),
                    stop=(kk == 1),
                )
            if act_copy and ob == 1:
                nc.scalar.copy(od[:, ob * 512 : (ob + 1) * 512], po_t[:])
            else:
                nc.vector.tensor_copy(od[:, ob * 512 : (ob + 1) * 512], po_t[:])
        (ring or nc.sync).dma_start(
            out=out_d[p * S + rc * 128 : p * S + (rc + 1) * 128, :], in_=od[:]
        )

    # ---- schedule -------------------------------------------------------
    # 8 windows of (p, pair m, side), sides ALTERNATING per pair; each
    # window's 16 exps (~18.4us) pace the kernel.  Fill work is slotted
    # between score chunks at ~13us PE per window.
    windows = [
        (0, 0, 0), (0, 0, 1), (0, 1, 0), (0, 1, 1),
        (1, 0, 0), (1, 0, 1), (1, 1, 0), (1, 1, 1),
    ]
    av_done = {}
    pending2 = []
    ex_tiles = {}

    def new_ex(widx):
        side = windows[widx][2]
        tag, dt = ("exb", EXL_DT) if side == 0 else ("ex8", EXR_DT)
        ex_tiles[widx] = [
            sb.tile([128, KC * 1024], dt, name=tag, tag=tag, bufs=2)
            for _ in range(2)
        ]

    def do_avl(widx, hi, qb):
        p, m, side = windows[widx]
        if qb == 0:
            av_done.setdefault(widx, {})[hi] = sb.tile(
                [65, S], F32, name="av", tag="avT", bufs=5
            )
        av_left_qb(av_done[widx][hi], p, m, hi, ex_tiles[widx][hi], qb)

    def do_av(widx, hi):
        p, m, side = windows[widx]
        av_done.setdefault(widx, {})[hi] = av_right(
            p, m, hi, ex_tiles[widx][hi]
        )

    def do_cp1(p, m, hi):
        wL = p * 4 + m * 2
        part2 = combine(
            p, 2 * m + hi, av_done[wL][hi], av_done[wL + 1][hi]
        )
        pending2.append(part2)

    def drain_one():
        pending2.pop(0)()

    # incremental right-path AV for the FINAL window (7): qb0 accumulates
    # in PSUM (tag av8acc, 2 banks) as exp chunks land; qb1 runs post-exp.
    av8i_acc = {}

    def av8i(widx, pr):
        p, m, side = windows[widx]
        pp = 1 - p
        v8r = v8[:].rearrange("p (rc x) -> p rc x", rc=16)
        for hi in (0, 1):
            h = 2 * m + hi
            if pr == 0:
                av8i_acc[hi] = ps.tile(
                    [65, 512], F32, name="av8i", tag="av8acc", bufs=2
                )
            exr = ex_tiles[widx][hi][:].rearrange("p (k n) -> p k n", k=KC)
            nc.tensor.matmul(
                av8i_acc[hi][:],
                v8r[:, pp * 8 + 2 * pr : pp * 8 + 2 * pr + 2, h * VW : h * VW + 65],
                exr[:, 2 * pr : 2 * pr + 2, 0:512],
                start=(pr == 0),
                stop=(pr == 3),
                perf_mode=DR,
            )

    def window(widx, fills):
        p, m, side = windows[widx]
        new_ex(widx)
        for kc in range(KC):
            qk_pair(p, m, side, ex_tiles[widx], (kc,))
            for f in fills[kc]:
                f()
        for f in fills[KC]:
            f()

    PC = lambda kind, rb, part: (lambda: proj_chunk(kind, rb, part))
    ST = lambda kind, rb, eng: (lambda: stage(kind, rb, eng))
    AV = lambda w, hi: (lambda: do_av(w, hi))
    AVL = lambda w, hi, qb: (lambda: do_avl(w, hi, qb))
    CP = lambda p, m, hi: (lambda: do_cp1(p, m, hi))
    AV8I = lambda w, pr: (lambda: av8i(w, pr))
    OP_ = lambda p, rc, ring: (lambda: outproj_rc(p, rc, ring=ring))

    # left-side chain for the final pair: runs during w7 (depends only on
    # AVL(6)); produces t1p + allocates the t2p/t3p tiles for the tail.
    tl = {}

    def tailL(hi):
        avL = av_done[6][hi]
        po = hi * 64
        rrL = sb.tile([1, S], F32, name="rrLt", tag="rrow", bufs=2)
        nc.vector.reciprocal(rrL[:], avL[64:65, :])
        bcL = sb.tile([64, S], F32, name="bcLt", tag="bc", bufs=1)
        nc.gpsimd.partition_broadcast(bcL[:], rrL[:])
        if hi == 0:
            tl["t1p"] = sb.tile([128, S], F32, name="t1p", tag="t1", bufs=1)
            tl["t2p"] = sb.tile([128, S], F32, name="t2p", tag="t2", bufs=1)
            tl["t3p"] = sb.tile([128, S], F32, name="t3p", tag="t3", bufs=1)
        nc.vector.tensor_tensor(
            out=tl["t1p"][po : po + 64, :], in0=avL[0:64, :], in1=bcL[:],
            op=OP.mult,
        )

    # ---- lead-in --------------------------------------------------------
    # critical stages split across the three DMA paths; warmup keeps the
    # PE clock high while they land
    stage("q", 0, nc.scalar)
    stage("q", 1, nc.gpsimd)
    stage("k", 0, nc.sync)
    stage("k", 1, nc.sync)
    nc.scalar.dma_start(out=wv_sb[:], in_=wv[:])
    warmup(WARMUP_N)
    nc.gpsimd.partition_broadcast(bv_bc[:], bv_row[:])
    nc.vector.tensor_scalar(
        out=mbias[:], in0=mask_sb[:], scalar1=-1, scalar2=MASK_NEG,
        op0=OP.add, op1=OP.mult,
    )
    proj_chunk("q", 0, 0)
    proj_chunk("q", 1, 0)
    proj_chunk("k", 0, 0)

    window(0, [  # (0,0,0) L
        [PC("k", 0, 1), ST("k", 2, nc.scalar), ST("v", 0, nc.gpsimd)],
        [PC("k", 1, 1)],
        [PC("k", 1, 0), ST("k", 3, nc.sync)],
        [PC("v", 0, 0)],
        [PC("v", 0, 1)],
        [PC("k", 2, 0)],
        [PC("k", 3, 0)],
        [PC("v", 0, 2)],
        [PC("v", 0, 3), ST("v", 1, nc.sync)],
    ])
    window(1, [  # (0,0,1) R
        [PC("q", 0, 1)],
        [PC("q", 1, 1)],
        [PC("v", 1, 0)],
        [PC("v", 1, 1)],
        [PC("v", 1, 2)],
        [PC("v", 1, 3), ST("v", 2, nc.gpsimd)],
        [AVL(0, 0, 0)],
        [AVL(0, 0, 1)],
        [],
    ])
    window(2, [  # (0,1,0) L
        [AVL(0, 1, 0), ST("q", 2, nc.scalar)],
        [AVL(0, 1, 1)],
        [PC("v", 2, 0)],
        [PC("v", 2, 1), lambda: nc.gpsimd.dma_start(out=wo_sb[:], in_=wo[:])],
        [PC("v", 2, 2)],
        [PC("v", 2, 3), ST("v", 3, nc.sync), ST("q", 3, nc.sync)],
        [PC("k", 2, 1)],
        [PC("k", 3, 1)],
        [],
    ])
    window(3, [  # (0,1,1) R
        [PC("v", 3, 0)],
        [PC("v", 3, 1)],
        [PC("v", 3, 2)],
        [PC("v", 3, 3)],
        [AV(1, 0)],
        [AV(1, 1)],
        [PC("q", 2, 0), CP(0, 0, 0)],
        [drain_one, CP(0, 0, 1), PC("q", 3, 0)],
        [drain_one],  # tanh (p0,m0) at the window boundary
    ])
    window(4, [  # (1,0,0) L
        [AVL(2, 0, 0)],
        [AVL(2, 0, 1)],
        [AVL(2, 1, 0)],
        [AVL(2, 1, 1)],
        [PC("q", 2, 1)],
        [AV(3, 0)],
        [AV(3, 1)],
        [CP(0, 1, 0), drain_one, CP(0, 1, 1)],
        [drain_one],  # tanh (p0,m1) at the window boundary
    ])
    window(5, [  # (1,0,1) R
        [PC("q", 3, 1)],
        [AVL(4, 0, 0)],
        [AVL(4, 0, 1)],
        [AVL(4, 1, 0)],
        [AVL(4, 1, 1)],
        [AV(5, 0)],
        [AV(5, 1)],
        [CP(1, 0, 0), drain_one, CP(1, 0, 1)],
        [drain_one],  # tanh (p1,m0) at the window boundary
    ])
    window(6, [  # (1,1,0) L
        [OP_(0, 0, nc.sync)],
        [OP_(0, 1, nc.gpsimd)],
        [OP_(0, 2, nc.sync)],
        [OP_(0, 3, nc.gpsimd)],
        [OP_(0, 4, nc.sync)],
        [OP_(0, 5, nc.gpsimd)],
        [OP_(0, 6, nc.sync)],
        [OP_(0, 7, nc.gpsimd)],
        [],
    ])
    window(7, [  # (1,1,1) R
        [AVL(6, 0, 0)],
        [AVL(6, 0, 1), AV8I(7, 0)],
        [lambda: tailL(0)],
        [AVL(6, 1, 0), AV8I(7, 1)],
        [AVL(6, 1, 1)],
        [lambda: tailL(1), AV8I(7, 2)],
        [],
        [],
        [AV8I(7, 3)],
    ])

    # ---- tail: qb-pipelined final combine + p1 output projection --------
    # qb0 right-path AVs accumulated incrementally during w7; here: copy
    # them out, run qb1 AVs, then per-qb combine -> tanh -> STT -> outproj.
    p_, m_ = 1, 1
    avR = {
        hi: sb.tile([65, S], F32, name="av", tag="avT", bufs=5) for hi in (0, 1)
    }
    v8r = v8[:].rearrange("p (rc x) -> p rc x", rc=16)

    def av8_qb1(hi):
        h = 2 * m_ + hi
        exr = ex_tiles[7][hi][:].rearrange("p (k n) -> p k n", k=KC)
        pa = ps.tile([65, 512], F32, name="av8i", tag="av8acc", bufs=2)
        for pr in range(4):
            nc.tensor.matmul(
                pa[:],
                v8r[:, 0 * 8 + 2 * pr : 0 * 8 + 2 * pr + 2, h * VW : h * VW + 65],
                exr[:, 2 * pr : 2 * pr + 2, 512:1024],
                start=(pr == 0),
                stop=(pr == 3),
                perf_mode=DR,
            )
        nc.vector.tensor_copy(avR[hi][:, 512:1024], pa[:])

    def tail_qb(qb):
        """Right-side normalize + tanh + STT for one q half of both heads."""
        cs = slice(qb * 512, (qb + 1) * 512)
        for hi in (0, 1):
            po = hi * 64
            rr = sb.tile([1, 512], F32, name="rrq", tag="rrow", bufs=2)
            nc.vector.reciprocal(rr[:], avR[hi][64:65, cs])
            bcp = sb.tile([64, 512], F32, name="bcq", tag="bcq", bufs=1)
            nc.gpsimd.partition_broadcast(bcp[:], rr[:])
            nc.vector.tensor_tensor(
                out=tl["t2p"][po : po + 64, cs], in0=avR[hi][0:64, cs],
                in1=bcp[:], op=OP.mult,
            )
        nc.scalar.activation(tl["t3p"][:, cs], tl["t2p"][:, cs], AF.Tanh)
        nc.vector.scalar_tensor_tensor(
            out=comb[1][:, S + qb * 512 : S + (qb + 1) * 512],
            in0=tl["t3p"][:, cs],
            scalar=0.1,
            in1=tl["t1p"][:, cs],
            op0=OP.mult,
            op1=OP.add,
        )

    # copy out the incrementally-accumulated qb0 AVs
    nc.vector.tensor_copy(avR[0][:, 0:512], av8i_acc[0][:])
    nc.vector.tensor_copy(avR[1][:, 0:512], av8i_acc[1][:])
    # qb1 AVs (PE) while the qb0 combine chain runs
    av8_qb1(0)
    tail_qb(0)
    av8_qb1(1)
    rings = [nc.sync, nc.scalar, nc.gpsimd]
    for rc in range(4):
        outproj_rc(1, rc, act_copy=True, ring=rings[rc % 3])
    tail_qb(1)
    for rc in range(4, 8):
        outproj_rc(1, rc, act_copy=True, ring=rings[rc % 3])

    if dbg is not None:
        nc.sync.dma_start(out=dbg["qT0"][:], in_=qT[0][:])
        nc.sync.dma_start(out=dbg["kT0"][:], in_=kT[0][:])
        nc.sync.dma_start(out=dbg["ex00"][:], in_=ex_tiles[6][0][:])
        nc.sync.dma_start(out=dbg["av00"][:], in_=av_done[0][0][:])
        nc.sync.dma_start(out=dbg["vaug"][:], in_=vb[:])
        nc.sync.dma_start(out=dbg["comb0"][:], in_=comb[0][:])


_CACHED = None


def _build():
    global _CACHED
    if _CACHED is not None:
        return _CACHED
    nc = bacc.Bacc("TRN2", target_bir_lowering=False, debug=False)
    tens = {
        "xq": nc.dram_tensor("xq", [4 * 128, KC * 512], X_DT, kind="ExternalInput"),
        "xk": nc.dram_tensor("xk", [4 * 128, KC * 512], X_DT, kind="ExternalInput"),
        "xv": nc.dram_tensor("xv", [4 * 128, KC * 512], X_DT, kind="ExternalInput"),
        "wq": nc.dram_tensor("wq", [128, KC * CH], X_DT, kind="ExternalInput"),
        "wk": nc.dram_tensor("wk", [128, KC * CH], X_DT, kind="ExternalInput"),
        "wv": nc.dram_tensor("wv", [128, KC * CH], X_DT, kind="ExternalInput"),
        "wo": nc.dram_tensor("wo", [128, 2 * D], OUT_DT, kind="ExternalInput"),
        "bq": nc.dram_tensor("bq", [CH], F32, kind="ExternalInput"),
        "bk": nc.dram_tensor("bk", [CH], F32, kind="ExternalInput"),
        "bv": nc.dram_tensor("bv", [CH], F32, kind="ExternalInput"),
        "mask": nc.dram_tensor("mask", [P, S], I32, kind="ExternalInput"),
    }
    out_d = nc.dram_tensor("out", [R, D], F32, kind="ExternalOutput")
    dbg = None
    if DEBUG:
        dbg = {
            "qT0": nc.dram_tensor("dbg_qT0", [128, R], QK_DT, kind="ExternalOutput"),
            "kT0": nc.dram_tensor("dbg_kT0", [128, R], QK_DT, kind="ExternalOutput"),
            "ex00": nc.dram_tensor("dbg_ex00", [128, KC * 1024], EXL_DT, kind="ExternalOutput"),
            "av00": nc.dram_tensor("dbg_av00", [65, S], F32, kind="ExternalOutput"),
            "vaug": nc.dram_tensor("dbg_vaug", [128, 16 * NH * 65], EXL_DT, kind="ExternalOutput"),
            "comb0": nc.dram_tensor("dbg_comb0", [128, R], OUT_DT, kind="ExternalOutput"),
        }
    with tile.TileContext(nc) as tc:
        _emit(nc, tc, tens, out_d, dbg=dbg)
    nc.compile()
    _CACHED = nc
    return nc


def _pack_x(xT, xnp):
    # xT: [D, R] -> [rb*128 + p, kc*512 + c] with d = kc*128+p, r = rb*512+c
    return np.ascontiguousarray(
        xT.reshape(KC, 128, 4, 512).transpose(2, 1, 0, 3).reshape(512, KC * 512)
    ).astype(xnp)


def _in_maps(query, key, value, mask, Wq, bq, Wk, bk, Wv, bv, Wo):
    xnp = mybir.dt.np(X_DT)
    onp = mybir.dt.np(OUT_DT)
    f32 = lambda a: np.ascontiguousarray(np.asarray(a, dtype=np.float32))
    query, key, value = f32(query), f32(key), f32(value)
    Wq, Wk, Wv, Wo = f32(Wq), f32(Wk), f32(Wv), f32(Wo)
    bq, bk, bv = f32(bq), f32(bk), f32(bv)
    mask = np.ascontiguousarray(np.asarray(mask, dtype=np.int32))

    xqP = [_pack_x(query[b].reshape(R, D).T, xnp) for b in range(B)]
    xkP = [_pack_x(key[b].reshape(R, D).T, xnp) for b in range(B)]
    xvP = [_pack_x(value[b].reshape(R, D).T, xnp) for b in range(B)]

    def pack_w(Wt):  # Wt: [D, CH] -> [p, kc*CH]
        return np.ascontiguousarray(
            Wt.reshape(KC, 128, CH).transpose(1, 0, 2).reshape(128, KC * CH)
        ).astype(xnp)

    maps = []
    for c in range(NCORES):
        b, hg = divmod(c, GROUP)
        ch = slice(hg * CH, (hg + 1) * CH)
        woT = Wo[:, ch].T  # [CH, D]
        maps.append(
            {
                "xq": xqP[b],
                "xk": xkP[b],
                "xv": xvP[b],
                "wq": pack_w(Wq[ch, :].T),
                "wk": pack_w(Wk[ch, :].T),
                "wv": pack_w(Wv[ch, :].T),
                "wo": np.ascontiguousarray(
                    woT.reshape(2, 128, D).transpose(1, 0, 2).reshape(128, 2 * D)
                ).astype(onp),
                "bq": bq[ch],
                "bk": bk[ch],
                "bv": bv[ch],
                "mask": mask[b, :, 0, :],
            }
        )
    return maps


def _run(in_maps, **kwargs):
    nc = _build()
    return run_bass_kernel_spmd(nc, in_maps, core_ids=list(range(NCORES)), **kwargs)


def kernel(query, key, value, mask, Wq, bq, Wk, bk, Wv, bv, Wo, bo):
    res = _run(_in_maps(query, key, value, mask, Wq, bq, Wk, bk, Wv, bv, Wo))
    bo = np.asarray(bo, dtype=np.float32)
    out = np.zeros((B, P, S, D), dtype=np.float32)
    for c in range(NCORES):
        b = c // GROUP
        out[b] += res.results[c]["out"].astype(np.float32).reshape(P, S, D)
    out += bo
    return out
